# revision 1
# baseline (speedup 1.0000x reference)
"""Trainium2 Bass kernel for a 3-layer binary-weight MLP.

Problem (nn_MLP_56779467653689):
    x: [8192, 1024] f32
    h = relu(s0 * (x @ W0)) * 2      W0 = 2*k0-1  in {-1,+1}, [1024, 4096]
    h = relu(s1 * (h @ W1)) * 2      W1 [4096, 4096]
    out = s2 * (h @ W2)              W2 [4096, 1024]

Strategy: data-parallel over tokens across 8 NeuronCores (1024/core),
activations [features, tokens] in SBUF. Layers 1 and 2 use one level of
Winograd-Strassen: C = A@B with A = Wt (weights, [out, in]) and B = H
([in, tok]) split 2x2; the 7 A-side combos have small-integer entries, so
they are precomputed on the host with the eviction scale folded in (exact
in bf16). The 4 B-side combos (T1..T4) and the 7 output combines (U-adds)
run on the otherwise-idle Vector engine; relu evictions on Scalar. PE
multiply count for L1/L2 drops to 7/8 of the naive kernel.

SBUF is tight: T2 overwrites B12's slots (B12 is dead after the T build),
T4 is produced into a small rotating pool just-in-time, and h2 reuses the
x slots (x is dead once layer 0 finishes).
"""

from contextlib import ExitStack

import ml_dtypes
import numpy as np

P = 128
TOKENS = 8192
D_IN = 1024
D_H = 4096
D_OUT = 1024
N_CORES = 8
TOK_PER_CORE = TOKENS // N_CORES  # 1024
TOK_TILE = 512
NT = TOK_PER_CORE // TOK_TILE  # 2

BF16 = ml_dtypes.bfloat16

TRACE = False
TRACE_CORES = None
LAST_EXEC_TIME_NS = None
LAST_RESULT = None

_cache = {}


def _prune_dma_waits(nc, max_waits=1):
    """Drop transitively-implied waits from DMA instructions.

    DMA queue-entry descriptors hold a single sync wait; Tile's sem
    assignment is per-proc minimal but not transitively minimal across
    procs, so a recycled SBUF slot's DMA can carry WAR (engine) + WAW
    (prev slot writer's DMA lane) + lane-recycle waits = 3. The WAW (and
    often the recycle) wait is implied by the engine wait: the readers
    counted by the WAR threshold themselves waited on those DMAs.

    Soundness: a wait (s >= v) on instruction I is dropped only when the
    completion clocks implied by I's *other* waits already guarantee
    cumulative increments of s reached v. Completion clocks are built
    forward over the scheduled BIR order giving same-stream predecessor
    credit only to in-order engines (PE/ACT/DVE/SP), never to DMA lanes
    or Pool. Unrecognized wait/update modes contribute no credit, so
    unknowns can only inhibit pruning, never enable it.
    """
    import bisect

    import bass_rust

    IN_ORDER_ENGINES = {
        "EngineType.PE",
        "EngineType.Activation",
        "EngineType.DVE",
        "EngineType.SP",
    }

    sem_hist = {}
    sem_cum = {}
    eng_clock = {}
    poisoned = set()

    def cc(sem, val):
        if sem in poisoned:
            return None
        hist = sem_hist.get(sem)
        if not hist or hist[0][-1] < val:
            return None
        return hist[1][bisect.bisect_left(hist[0], val)]

    def merge(dst, src):
        for k, v in src.items():
            if dst.get(k, 0) < v:
                dst[k] = v

    pruned = 0
    for bb in nc.m.functions[0].blocks:
        for inst in bb.instructions:
            si = inst.sync_info
            waits = list(si.on_wait or []) if si is not None else []
            ups = list(si.on_update or []) if si is not None else []
            is_dma = type(inst).__name__ == "InstDMACopy"

            clock = {}
            if not is_dma:
                prev = eng_clock.get(str(inst.engine))
                if prev is not None and str(inst.engine) in IN_ORDER_ENGINES:
                    merge(clock, prev)
            for w in waits:
                if w.wait_mode == "sem-ge-imm" and w.wait_value is not None:
                    c = cc(w.ant_name, w.wait_value)
                    if c is not None:
                        merge(clock, c)

            tname = type(inst).__name__
            if is_dma:
                cap = max_waits
            elif tname in ("InstDrain", "InstEventSemaphore", "InstCall",
                           "InstUnconditionalBranch", "InstISA"):
                cap = None
            else:
                cap = 2

            if cap is not None and len(waits) > cap:
                kept = list(waits)
                changed = True
                while len(kept) > cap and changed:
                    changed = False
                    for w in list(kept):
                        if w.wait_mode != "sem-ge-imm" or w.wait_value is None:
                            continue
                        implied = {}
                        provable = True
                        for o in kept:
                            if o is w:
                                continue
                            if o.wait_mode != "sem-ge-imm" or o.wait_value is None:
                                provable = False
                                break
                            c = cc(o.ant_name, o.wait_value)
                            if c is None:
                                provable = False
                                break
                            merge(implied, c)
                        if provable and implied.get(w.ant_name, 0) >= w.wait_value:
                            kept.remove(w)
                            pruned += 1
                            changed = True
                            break
                if len(kept) != len(waits):
                    inst.sync_info = bass_rust.SyncInfo(on_wait=kept, on_update=ups)

            own = {}
            for u in ups:
                if u.update_mode not in ("sem-inc", "sem-add-imm"):
                    poisoned.add(u.ant_name)
                    continue
                inc = 1 if u.update_mode == "sem-inc" else u.update_value
                if inc is None:
                    poisoned.add(u.ant_name)
                    continue
                sem = u.ant_name
                sem_cum[sem] = sem_cum.get(sem, 0) + inc
                own[sem] = sem_cum[sem]
            merge(clock, own)
            for sem, cum in own.items():
                vals, clocks = sem_hist.setdefault(sem, ([], []))
                vals.append(cum)
                clocks.append(clock)
            if not is_dma:
                eng_clock[str(inst.engine)] = clock
    return pruned


def _build(a0):
    """SPMD single-core program. Layer-1/2 eviction scales are folded into
    the host-packed Strassen weight combos; only layer 0's scale (a0) is a
    runtime constant here."""
    import concourse.mybir as mybir
    import concourse.tile as tile
    from concourse import bacc

    nc = bacc.Bacc(
        "TRN2",
        target_bir_lowering=False,
        debug=False,
        enable_asserts=False,
        num_devices=N_CORES,
    )
    bf = mybir.dt.bfloat16
    f32 = mybir.dt.float32

    xt = nc.dram_tensor("xt", [NT, P, D_IN // P * TOK_TILE], bf,
                        kind="ExternalInput")
    w0p = nc.dram_tensor("w0p", [D_H // P, P, D_IN], bf, kind="ExternalInput")
    w1s = nc.dram_tensor("w1s", [7, 16, P, D_H // 2], bf, kind="ExternalInput")
    w2s = nc.dram_tensor("w2s", [7, 4, P, D_H // 2], bf, kind="ExternalInput")
    outt = nc.dram_tensor("outt", [D_OUT, TOK_PER_CORE], bf, kind="ExternalOutput")

    relu = mybir.ActivationFunctionType.Relu

    with tile.TileContext(nc) as tc, ExitStack() as ctx:
        flat = ctx.enter_context(tc.tile_pool(name="flat", bufs=1))
        wpool = ctx.enter_context(tc.tile_pool(name="wp", bufs=8))
        upool = ctx.enter_context(tc.tile_pool(name="up", bufs=1))
        mpool = ctx.enter_context(tc.tile_pool(name="mp", bufs=2))
        tpool = ctx.enter_context(tc.tile_pool(name="tp", bufs=8))
        spool = ctx.enter_context(tc.tile_pool(name="sp", bufs=4))
        opool = ctx.enter_context(tc.tile_pool(name="op", bufs=4))
        pspool = ctx.enter_context(tc.tile_pool(name="psp", bufs=8, space="PSUM"))

        def ftile(tag):
            return flat.tile([P, TOK_TILE], bf, tag=tag, name=tag)

        # ---- layer 0: naive. x is host-packed to [t, 128, j*512+c] so each
        # half loads as two wide DMAs (4KB row chunks) split across queues.
        xbig = []
        for t in range(NT):
            xb = flat.tile([P, D_IN // P * TOK_TILE], bf, tag=f"xb_{t}",
                           name=f"xb_{t}")
            for j in range(D_IN // P):
                q = nc.scalar if j % 2 == 0 else nc.gpsimd
                q.dma_start(
                    out=xb[:, j * TOK_TILE : (j + 1) * TOK_TILE],
                    in_=xt[t, :, j * TOK_TILE : (j + 1) * TOK_TILE],
                )
            xbig.append(xb)

        def x_sl(j, t):
            return xbig[t][:, j * TOK_TILE : (j + 1) * TOK_TILE]

        # h1 tile tags: (s, t) for s in 0..31; (s<16, t=1) lives in "b12" tags
        def h1tag(s, t):
            return f"b12_{s}" if (s < 16 and t == 1) else f"h1_{s}_{t}"

        h1 = {}
        w0tiles = {}

        def l0_chain(n, t):
            if n not in w0tiles:
                w = wpool.tile([P, D_IN], bf, tag="w", name=f"w0_{n}")
                nc.sync.dma_start(out=w[:], in_=w0p[n])
                w0tiles[n] = w
            w = w0tiles[n]
            ps = pspool.tile([P, TOK_TILE], f32, tag="ps", name=f"ps0_{n}_{t}")
            for j in range(D_IN // P):
                nc.tensor.matmul(
                    ps[:],
                    w[:, j * P : (j + 1) * P],
                    x_sl(j, t),
                    start=(j == 0),
                    stop=(j == D_IN // P - 1),
                )
            ht = ftile(h1tag(n, t))
            nc.scalar.activation(ht[:], ps[:], relu, scale=a0)
            h1[(n, t)] = ht

        # First strips run token-half 0 only, buying arrival slack for the
        # second half of x; then the normal (n, t) sweep.
        l0_order = [(n, 0) for n in range(4)] + [(n, 1) for n in range(4)]
        l0_order += [(n, t) for n in range(4, D_H // P) for t in range(NT)]
        for n, t in l0_order:
            l0_chain(n, t)

        # ---- h2 slot map: first 16 tiles reuse the xb region (x is dead
        # once layer 0 finishes, before any h2 write) ----
        h2_order = []
        for g in range(16):
            h2_order += [(g, 0), (16 + g, 0), (g, 1), (16 + g, 1)]
        h2map = {}
        for i, st in enumerate(h2_order):
            if i < 16:
                h2map[st] = xbig[i // 8][:, (i % 8) * TOK_TILE :
                                         (i % 8 + 1) * TOK_TILE]
            else:
                h2map[st] = ftile(f"h2_{i - 16}")[:]

        def strassen(nks, ngrp, w_dram, bsel, emit, tbuild, make_t4):
            """One Strassen level: for each output-strip group, run the 7
            Mi chains (each contracting nks K-tiles) and combine.

            bsel(i, j) -> AP of B-block i's j-th strip; emit(g, name, ap_or
            psum...) handled inline below instead; tbuild() issues the
            T-combo builds once."""
            tbuild()
            for g in range(ngrp):
                pss = {}

                def chain(*idxs):
                    # interleaved chains alternate PSUM banks between MMs
                    tiles = []
                    for i in idxs:
                        wA = wpool.tile([P, 1024], bf, tag="w", name=f"w_{g}_{i}a")
                        wB = wpool.tile([P, 1024], bf, tag="w", name=f"w_{g}_{i}b")
                        nc.sync.dma_start(out=wA[:], in_=w_dram[i, g, :, 0:1024])
                        nc.gpsimd.dma_start(
                            out=wB[:], in_=w_dram[i, g, :, 1024:2048]
                        )
                        ps = pspool.tile(
                            [P, TOK_TILE], f32, tag="ps", name=f"ps_{g}_{i}"
                        )
                        pss[i] = ps
                        tiles.append((i, wA, wB, ps))
                    for j in range(nks):
                        for i, wA, wB, ps in tiles:
                            w = wA if j < 8 else wB
                            jj = j if j < 8 else j - 8
                            rhs = make_t4(g, j)[:] if i == 3 else bsel(i, j)
                            nc.tensor.matmul(
                                ps[:],
                                w[:, jj * P : (jj + 1) * P],
                                rhs,
                                start=(j == 0),
                                stop=(j == nks - 1),
                            )

                # Mi chains: 0:B11 4:T1 6:T3 1:B21 2:B22 5:T2 3:T4
                chain(0, 4)
                m1s = mpool.tile([P, TOK_TILE], f32, tag="m1s", name=f"m1s_{g}")
                nc.scalar.copy(m1s[:], pss[0][:])
                chain(6, 1)
                emit(g, "c11", m1s, pss[1], False)
                chain(2, 5)
                u2 = upool.tile([P, TOK_TILE], f32, tag="u2", name=f"u2_{g}")
                nc.vector.tensor_add(u2[:], m1s[:], pss[5][:])
                u4 = upool.tile([P, TOK_TILE], f32, tag="u4", name=f"u4_{g}")
                nc.vector.tensor_add(u4[:], u2[:], pss[4][:])
                u3 = upool.tile([P, TOK_TILE], f32, tag="u3", name=f"u3_{g}")
                nc.vector.tensor_add(u3[:], u2[:], pss[6][:])
                emit(g, "c22", u3, pss[4], False)
                emit(g, "c12", u4, pss[2], False)
                chain(3)
                emit(g, "c21", u3, pss[3], True)

        # ---- layer 1 strassen ----
        t1 = {}
        t3 = {}
        t2 = {}

        def tbuild1():
            for s in range(16):
                t1[s] = ftile(f"t1_{s}")
                nc.vector.tensor_sub(t1[s][:], h1[(s, 1)][:], h1[(s, 0)][:])
                t3[s] = ftile(f"t3_{s}")
                nc.vector.tensor_sub(t3[s][:], h1[(16 + s, 1)][:], h1[(s, 1)][:])
            for s in range(16):
                t2[s] = ftile(f"b12_{s}")  # overwrite B12 (dead after T1/T3)
                nc.vector.tensor_sub(t2[s][:], h1[(16 + s, 1)][:], t1[s][:])

        def bsel1(i, j):
            if i == 0:
                return h1[(j, 0)][:]
            if i == 1:
                return h1[(16 + j, 0)][:]
            if i == 2:
                return h1[(16 + j, 1)][:]
            if i == 4:
                return t1[j][:]
            if i == 5:
                return t2[j][:]
            return t3[j][:]

        def make_t4_1(g, j):
            t4 = tpool.tile([P, TOK_TILE], bf, tag="t4", name=f"t4_{g}_{j}")
            nc.vector.tensor_sub(t4[:], t2[j][:], h1[(16 + j, 0)][:])
            return t4

        h2 = {}

        def emit1(g, cname, sb, ps, is_sub):
            st = {"c11": (g, 0), "c21": (16 + g, 0), "c12": (g, 1),
                  "c22": (16 + g, 1)}[cname]
            c = spool.tile([P, TOK_TILE], bf, tag="st", name=f"c_{g}_{cname}")
            if is_sub:
                nc.vector.tensor_sub(c[:], sb[:], ps[:])
            else:
                nc.vector.tensor_add(c[:], sb[:], ps[:])
            ht = h2map[st]
            nc.scalar.activation(ht, c[:], relu)
            h2[st] = ht

        strassen(16, 16, w1s, bsel1, emit1, tbuild1, make_t4_1)

        # ---- layer 2 strassen (no relu; outputs DMA'd as they complete) ----
        t1b = {}
        t3b = {}
        t2b = {}

        def tbuild2():
            # T tiles reuse h1's slots (h1 dead after layer 1's chains)
            for s in range(16):
                t1b[s] = ftile(f"h1_{s}_0")
                nc.vector.tensor_sub(t1b[s][:], h2[(s, 1)], h2[(s, 0)])
                t3b[s] = ftile(f"h1_{16 + s}_0")
                nc.vector.tensor_sub(t3b[s][:], h2[(16 + s, 1)], h2[(s, 1)])
            for s in range(16):
                t2b[s] = ftile(f"h1_{16 + s}_1")
                nc.vector.tensor_sub(t2b[s][:], h2[(16 + s, 1)], t1b[s][:])

        def bsel2(i, j):
            if i == 0:
                return h2[(j, 0)]
            if i == 1:
                return h2[(16 + j, 0)]
            if i == 2:
                return h2[(16 + j, 1)]
            if i == 4:
                return t1b[j][:]
            if i == 5:
                return t2b[j][:]
            return t3b[j][:]

        def make_t4_2(g, j):
            t4 = tpool.tile([P, TOK_TILE], bf, tag="t4", name=f"t4b_{g}_{j}")
            nc.vector.tensor_sub(t4[:], t2b[j][:], h2[(16 + j, 0)])
            return t4

        def emit2(g, cname, sb, ps, is_sub):
            row, t = {"c11": (g, 0), "c21": (4 + g, 0), "c12": (g, 1),
                      "c22": (4 + g, 1)}[cname]
            o = opool.tile([P, TOK_TILE], bf, tag="o", name=f"o_{g}_{cname}")
            if is_sub:
                nc.vector.tensor_sub(o[:], sb[:], ps[:])
            else:
                nc.vector.tensor_add(o[:], sb[:], ps[:])
            q = nc.scalar if t == 0 else nc.gpsimd
            q.dma_start(
                out=outt[row * P : (row + 1) * P,
                         t * TOK_TILE : (t + 1) * TOK_TILE],
                in_=o[:],
            )

        strassen(16, 4, w2s, bsel2, emit2, tbuild2, make_t4_2)

    _prune_dma_waits(nc)
    nc.finalize()
    return nc


def _pack_w(k):
    """Bool [K, N] -> bf16 +-1 packed [N/P, P, K]."""
    K, N = k.shape
    w = np.where(k, np.float32(1.0), np.float32(-1.0)).astype(BF16)
    return np.ascontiguousarray(
        w.reshape(K // P, P, N // P, P).transpose(2, 1, 0, 3).reshape(N // P, P, K)
    )


def _pack_f(w):
    """Float [K, N] -> bf16 packed [N/P, P, K] (strip, partition=K, free)."""
    K, N = w.shape
    w = np.ascontiguousarray(w).astype(BF16)
    return np.ascontiguousarray(
        w.reshape(K // P, P, N // P, P).transpose(2, 1, 0, 3).reshape(N // P, P, K)
    )


def _strassen_weights(kmat, alpha):
    """Weight-side Winograd combos for C = Wt @ H, Wt = (2k-1).T scaled by
    alpha (folded eviction scale; exact in bf16 for power-of-two alpha).
    Returns [7, M/2/P, P, K/2] with Mi order [A11, A12, S4, A22, S1, S2, S3]
    matching B order [B11, B21, B22, T4, T1, T2, T3]."""
    Wt = np.where(kmat, 1.0, -1.0).astype(np.float32).T
    M, K = Wt.shape
    mh, kh = M // 2, K // 2
    A11, A12 = Wt[:mh, :kh], Wt[:mh, kh:]
    A21, A22 = Wt[mh:, :kh], Wt[mh:, kh:]
    S1 = A21 + A22
    S2 = S1 - A11
    S3 = A11 - A21
    S4 = A12 - S2
    packs = []
    for A in (A11, A12, S4, A22, S1, S2, S3):
        packs.append(_pack_f((alpha * A).T))
    return np.ascontiguousarray(np.stack(packs))


def _enable_ntff_trace():
    """Best-effort plumbing for trace=True under axon in this image."""
    import sys
    import types

    import concourse.bass_utils as bu

    bu.upload_artifacts = lambda tmpdir: tmpdir
    try:
        from antenv import axon_hooks
    except ImportError:
        import antenv

        axon_hooks = types.ModuleType("antenv.axon_hooks")
        _state = {"hook": None}
        axon_hooks.set_axon_ntff_profile_hook = lambda h: _state.__setitem__(
            "hook", h
        )
        axon_hooks.get_axon_ntff_profile_hook = lambda: _state["hook"]
        sys.modules["antenv.axon_hooks"] = axon_hooks
        antenv.axon_hooks = axon_hooks
    if axon_hooks.get_axon_ntff_profile_hook() is None:
        from trn_agent_boot.trn_boot import _ntff_profile_via_ctypes

        axon_hooks.set_axon_ntff_profile_hook(
            _ntff_profile_via_ctypes("/opt/axon/libaxon_pjrt.so")
        )


def kernel(x, k0, k1, k2, s0, s1, s2):
    global LAST_EXEC_TIME_NS, LAST_RESULT
    from concourse.bass_utils import run_bass_kernel_spmd

    if TRACE:
        _enable_ntff_trace()

    x = np.asarray(x)
    a0 = 2.0 * float(np.asarray(s0))
    a1 = 2.0 * float(np.asarray(s1))
    a2 = float(np.asarray(s2))

    key = (a0,)
    if key not in _cache:
        _cache[key] = _build(a0)
    nc = _cache[key]

    w0p = _pack_w(np.asarray(k0))
    w1s = _strassen_weights(np.asarray(k1), a1)
    w2s = _strassen_weights(np.asarray(k2), a2)

    in_maps = []
    for i in range(N_CORES):
        xs = x[i * TOK_PER_CORE : (i + 1) * TOK_PER_CORE].astype(BF16)
        xsT = np.ascontiguousarray(xs.T)  # [feat, tok]
        xp = np.ascontiguousarray(
            xsT.reshape(D_IN // P, P, NT, TOK_TILE)
            .transpose(2, 1, 0, 3)
            .reshape(NT, P, D_IN // P * TOK_TILE)
        )
        in_maps.append(
            {
                "xt": xp,
                "w0p": w0p,
                "w1s": w1s,
                "w2s": w2s,
            }
        )

    res = run_bass_kernel_spmd(
        nc, in_maps, list(range(N_CORES)), trace=TRACE, trace_cores=TRACE_CORES
    )
    LAST_EXEC_TIME_NS = res.exec_time_ns
    LAST_RESULT = res
    out = np.concatenate(
        [res.results[i]["outt"].T.astype(np.float32) for i in range(N_CORES)],
        axis=0
    )
    return np.ascontiguousarray(out)

